# revision 13
# baseline (speedup 1.0000x reference)
"""Trainium2 Bass kernel: batched single-head self-attention.

Reference computation (per (b, l) pair, 20 independent blocks):
    X = x[b, l] viewed as [N=1024, D=256] (xf layout)
    out[b, l] = softmax(beta * X @ X.T, axis=-1) @ X

Device algorithm (per block):
  * Scores: S[m, n] = sum_d X^T[d, m] X^T[d, n] on the TensorEngine with
    D on partitions.  All matmul MOVING operands are bf16 (PE streams
    bf16 at 1 cyc/col; fp32r takes ~2).  bf16 everywhere costs ~7e-3
    rel-max error on this data vs the 2e-2 gate (fp64-oracle verified).
  * Softmax shift W = exp(beta*(S - c_n)): the per-query -c row arrives
    as a 12KB DRAM row, is replicated across partitions by GpSimd
    partition_broadcast, and is added to the PSUM scores by
    scalar_tensor_tensor ops ALTERNATING between VectorE and GpSimd
    (one engine alone cannot keep up with the PE).  ScalarE then exps
    the shifted fp32 tile straight to bf16 W and does nothing else.
  * Second matmul: O[n, d] = sum_m W[m, n] xfo[m, d] with the W slice
    [128, 128] STATIONARY and xfo[m, 0:258] = [x | 1 | 0] moving, so
    every streamed column feeds 128 output rows and Z_n falls out as
    output column 256.  No separate Z pass exists.
  * The work is organized as two h-SWEEPS per block (query halves):
    sweep h computes the 8 score tiles [128k x 512q] and the 4 O
    accumulators for its query half, O trailing scores by 3 key tiles.
    This maps the 8-bank PSUM exactly (4 score ring + 4 O accumulators)
    and means sweep 0 only needs the first halves of xb -- which drives
    the DMA plan below.
  * The three DMA queues (Sync/Activation/Pool) run at only ~45GB/s
    each, so input transfers are split into chunks issued in
    consumption order: xb slab0 quarters first (the 2 queues' heads),
    then the -c row, then xfo0 / later slabs.  Outputs are written as
    bf16 (halves the out traffic; +~2e-3 error) and each 2-tile
    evacuation pair is chased by its own DMA on a per-slab queue.

Host pre/post (layout + O(N*D) work only; all O(N^2*D) flops on device):
  * xb   = X^T in bf16                  (score operands)
  * xf   = [X | 1 | 0] in bf16          (value operand)
  * nb   = -||x_n||^2 as one fp32 row   (shift, replicated on device)
  * out  = O[:, :256] / O[:, 256:257]   (normalize; already [n, d])

Sharding: 20 blocks over 8 cores as 2 full blocks + 1 half block (512
queries) per core -- exact, no padded compute.  The half blocks use a
host-side rotation of the key axis so every core runs the identical
program (softmax is invariant to key permutation when values are
permuted identically).
"""

import numpy as np
import ml_dtypes

import concourse.tile as tile
from concourse import bacc, mybir
from concourse.bass_utils import run_bass_kernel_spmd

F32 = mybir.dt.float32
BF16 = mybir.dt.bfloat16

B, L, D, H, W = 4, 5, 256, 32, 32
N = H * W            # 1024 keys per block
NBLK = B * L         # 20
NCORES = 8
NFULL = 2            # full blocks per core
NSLAB = 3            # 2 full + 1 half
DF = 272             # value operand row: [x | 1 | 0 | pad...] -- padded so
                     # bf16 rows stay 32B-aligned (272*2 = 544 = 17*32)
DO = 258             # O matmul moving width / output row: [d0..d255, Z, 0]

EXP = mybir.ActivationFunctionType.Exp
ALU = mybir.AluOpType


def build_program(beta: float):
    nc = bacc.Bacc("TRN2", target_bir_lowering=False, debug=False,
                   num_devices=NCORES)
    xb_in = nc.dram_tensor("xb_in", [NSLAB, 128, 2, N], BF16,
                           kind="ExternalInput")
    xf_in = nc.dram_tensor("xf_in", [NSLAB, 128, 8, DF], BF16,
                           kind="ExternalInput")
    nb_in = nc.dram_tensor("nb_in", [1, NSLAB * N], F32,
                           kind="ExternalInput")
    y_out = nc.dram_tensor("y_out", [NSLAB, 128, 8, DO], BF16,
                           kind="ExternalOutput")

    with tile.TileContext(nc) as tc:
        _build(tc, nc, xb_in.ap(), xf_in.ap(), nb_in.ap(), y_out.ap(), beta)
    nc.finalize()
    return nc


def _build(tc, nc, xb_in, xf_in, nb_in, y_out, beta):
    import contextlib
    ctx = contextlib.ExitStack()
    with ctx:
        const = ctx.enter_context(tc.tile_pool(name="const", bufs=1))
        xb_pool = ctx.enter_context(tc.tile_pool(name="xb", bufs=NSLAB))
        xfo_pool = ctx.enter_context(tc.tile_pool(name="xfo", bufs=NSLAB))
        nb_pool = ctx.enter_context(tc.tile_pool(name="nb", bufs=1))
        ssh_pool = ctx.enter_context(tc.tile_pool(name="ssh", bufs=6))
        # W tiles stay live until the h1 sweep at the end of the block.
        w_pool = ctx.enter_context(tc.tile_pool(name="w", bufs=10))
        o_sb_pool = ctx.enter_context(tc.tile_pool(name="o_sb", bufs=2))
        # PSUM: 4-deep score ring + 4 O accumulators = 8 banks.
        ps_s = ctx.enter_context(tc.tile_pool(name="ps_s", bufs=4, space="PSUM"))
        ps_o = ctx.enter_context(tc.tile_pool(name="ps_o", bufs=4, space="PSUM"))

        # Warm the PE clock (HAM) during the input-DMA window --
        # otherwise the first ~4us of real matmuls run at reduced clock.
        warm_src = const.tile([128, 512], F32)
        nc.gpsimd.memset(warm_src[:], 0.0)
        for wi in range(2):
            warm_ps = ps_o.tile([128, 512], F32, tag="o", name=f"warm_{wi}")
            nc.tensor.matmul(warm_ps[:], warm_src[:, 0:128], warm_src[:],
                             start=True, stop=True)

        # ---- input DMA plan ---------------------------------------
        # The 3 DMA queues are ~45GB/s each; chunks are issued in
        # consumption order so nothing early waits on late bytes.
        #   sync:   xb0[c0,h0] xb0[c0,h1] xb1 xb2     (+ y slab1)
        #   scalar: xb0[c1,h0] nb_row xb0[c1,h1] xfo1 (+ y slab2)
        #   gpsimd: xfo0 xfo2                         (+ y slab0)
        xb0 = xb_pool.tile([128, 2, N], BF16, tag="xb", name="xb_0")
        nc.sync.dma_start(out=xb0[:, 0, 0:512], in_=xb_in[0][:, 0, 0:512])
        nc.scalar.dma_start(out=xb0[:, 1, 0:512], in_=xb_in[0][:, 1, 0:512])
        nb_row = nb_pool.tile([1, NSLAB * N], F32, tag="nb_row")
        nc.scalar.dma_start(out=nb_row[:], in_=nb_in[:])
        nc.sync.dma_start(out=xb0[:, 0, 512:N], in_=xb_in[0][:, 0, 512:N])
        nc.scalar.dma_start(out=xb0[:, 1, 512:N], in_=xb_in[0][:, 1, 512:N])
        xfo0 = xfo_pool.tile([128, 8, DF], BF16, tag="xfo", name="xfo_0")
        for a0 in range(0, 8, 2):
            # per-pair chunks: O matmuls for key tile a gate on chunk
            # a//2 rather than on the whole 544KB transfer
            nc.gpsimd.dma_start(out=xfo0[:, a0:a0 + 2, :],
                                in_=xf_in[0][:, a0:a0 + 2, :])
        xb1 = xb_pool.tile([128, 2, N], BF16, tag="xb", name="xb_1")
        nc.sync.dma_start(out=xb1[:], in_=xb_in[1])
        xfo1 = xfo_pool.tile([128, 8, DF], BF16, tag="xfo", name="xfo_1")
        nc.scalar.dma_start(out=xfo1[:], in_=xf_in[1])
        xb2 = xb_pool.tile([128, 2, N], BF16, tag="xb", name="xb_2")
        nc.sync.dma_start(out=xb2[:], in_=xb_in[2])
        xfo2 = xfo_pool.tile([128, 8, DF], BF16, tag="xfo", name="xfo_2")
        nc.gpsimd.dma_start(out=xfo2[:], in_=xf_in[2])
        xbs, xfos = [xb0, xb1, xb2], [xfo0, xfo1, xfo2]
        y_queues = [nc.gpsimd, nc.sync, nc.scalar]

        # replicate -c to all 128 partitions, 512 queries at a time in
        # consumption order, on the (otherwise idle) GpSimd engine
        nb_all = nb_pool.tile([128, NSLAB * N], F32, tag="nb")
        for s in range(NSLAB):
            for h in range(2 if s < NFULL else 1):
                cs = slice(s * N + h * 512, s * N + (h + 1) * 512)
                nc.gpsimd.partition_broadcast(nb_all[:, cs], nb_row[:, cs])

        for s in range(NSLAB):
            n_q = N if s < NFULL else N // 2
            n_sw = n_q // 512   # h sweeps (1 or 2)
            xb, xfo = xbs[s], xfos[s]

            wt_tiles = [w_pool.tile([128, N], BF16, tag="w",
                                    name=f"w_{s}_{a}") for a in range(8)]
            o_sb = o_sb_pool.tile([128, 8, DO], BF16, tag="o_sb")

            for h in range(n_sw):
                hs = slice(h * 512, (h + 1) * 512)
                o_tiles = {}

                def scores(a):
                    asl = slice(a * 128, (a + 1) * 128)
                    sp = ps_s.tile([128, 512], F32, tag="sps",
                                   name=f"sps_{s}_{h}_{a}")
                    for c in range(2):
                        nc.tensor.matmul(sp[:], xb[:, c, asl],
                                         xb[:, c, hs],
                                         start=(c == 0), stop=(c == 1))
                    # shift: s_sh = S + (-c_n) on the VectorE (GpSimd
                    # cannot read PSUM; with one tile per sweep step the
                    # VectorE keeps up with the PE on its own)
                    ssh = ssh_pool.tile([128, 512], F32, tag="ssh",
                                        name=f"ssh_{s}_{h}_{a}")
                    nc.vector.scalar_tensor_tensor(
                        ssh[:], sp[:], 1.0,
                        nb_all[:, s * N + h * 512: s * N + (h + 1) * 512],
                        ALU.mult, ALU.add)
                    # W = exp(beta * s_sh) -> bf16; ScalarE does only exp
                    nc.scalar.activation(wt_tiles[a][:, hs], ssh[:], EXP,
                                         scale=float(beta))

                def emit_o(a):
                    # O[q] += W[a][:, q].T @ xfo[a]; column 256 = Z
                    for q in range(4 * h, 4 * h + 4):
                        if a == 0:
                            o_tiles[q] = ps_o.tile([128, DO], F32, tag="o",
                                                   name=f"o_{s}_{h}_{q}")
                        qs = slice(q * 128, (q + 1) * 128)
                        nc.tensor.matmul(o_tiles[q][:], wt_tiles[a][:, qs],
                                         xfo[:, a, 0:DO],
                                         start=(a == 0), stop=(a == 7))

                for a in range(8):
                    scores(a)
                    if a >= 3:
                        emit_o(a - 3)
                for a in (5, 6, 7):
                    emit_o(a)
                # evacuate to bf16 in pairs (DVE + ACT), each pair
                # chased by its output DMA on this slab's queue
                for p0 in range(4 * h, 4 * h + 4, 2):
                    nc.vector.tensor_copy(o_sb[:, p0, :], o_tiles[p0][:])
                    nc.scalar.copy(o_sb[:, p0 + 1, :], o_tiles[p0 + 1][:])
                    y_queues[s].dma_start(out=y_out[s][:, p0:p0 + 2, :],
                                          in_=o_sb[:, p0:p0 + 2, :])


_PROG_CACHE = {}


def _get_program(beta: float):
    if beta not in _PROG_CACHE:
        _PROG_CACHE[beta] = build_program(beta)
    return _PROG_CACHE[beta]


def make_in_maps(x: np.ndarray):
    """Shard the full input [B, L, D, H, W] into 8 per-core input maps."""
    xt_all = np.ascontiguousarray(x.reshape(NBLK, D, N))
    in_maps = []
    for c in range(NCORES):
        half_blk = NFULL * NCORES + c // 2
        half = xt_all[half_blk]
        if c % 2 == 1:
            # rotate keys so this core's queries are columns 0..511
            half = np.concatenate([half[:, N // 2:], half[:, :N // 2]], axis=1)
        slabs = np.stack([xt_all[NFULL * c], xt_all[NFULL * c + 1], half])
        xf = np.zeros((NSLAB, N, DF), np.float32)
        xf[:, :, :D] = slabs.transpose(0, 2, 1)
        xf[:, :, D] = 1.0
        negc = -np.einsum('sdn,sdn->sn', slabs, slabs)
        # pack into device layout: xb [128, 2, N], xf [128, 8, DF]
        xb_p = slabs.reshape(NSLAB, 2, 128, N).transpose(0, 2, 1, 3)
        xf_p = xf.reshape(NSLAB, 8, 128, DF).transpose(0, 2, 1, 3)
        in_maps.append({
            "xb_in": np.ascontiguousarray(xb_p.astype(ml_dtypes.bfloat16)),
            "xf_in": np.ascontiguousarray(xf_p.astype(ml_dtypes.bfloat16)),
            "nb_in": np.ascontiguousarray(negc.reshape(1, NSLAB * N)),
        })
    return in_maps


def assemble_output(results):
    """Normalize and gather per-core outputs into [B, L, N, D]."""
    out = np.empty((NBLK, N, D), np.float32)
    for c in range(NCORES):
        # y [NSLAB, 128, 8, DO]: [q-within-tile, q-tile, feature]
        y = (results[c]["y_out"].astype(np.float32)
             .transpose(0, 2, 1, 3).reshape(NSLAB, N, DO))
        for s, blk, lo, n_q in ((0, NFULL * c, 0, N),
                                (1, NFULL * c + 1, 0, N),
                                (2, NFULL * NCORES + c // 2,
                                 (c % 2) * (N // 2), N // 2)):
            o = y[s, :n_q]
            out[blk, lo:lo + n_q] = o[:, :D] / o[:, D:D + 1]
    return out.reshape(B, L, N, D)


def kernel(x, beta, _trace=False, _fast=True):
    x = np.asarray(x, dtype=np.float32)
    assert x.shape == (B, L, D, H, W), x.shape
    beta_f = float(np.asarray(beta))
    prog = _get_program(beta_f)
    in_maps = make_in_maps(x)
    res = run_bass_kernel_spmd(prog, in_maps, core_ids=list(range(NCORES)),
                               trace=_trace)
    out = assemble_output(res.results)
    if _trace:
        return out, res
    return out
